# revision 19
# baseline (speedup 1.0000x reference)
"""Skip-gram negative-sampling loss on 8 Trainium2 NeuronCores.

Strategy v2 (data-parallel over batch, hint-conformant):
  - Each core handles 2048 batch rows and 512 hierarchy pairs.
  - Row-major pair layout: block b (of 16) covers batch rows
    [128b, 128b+128); pair (row, j) sits at partition row%128, slot j
    (j<10: pos, j>=10: neg). The input-row operand is gathered ONCE per
    core (2048 rows) and broadcast (stride-0) across the 60 slots by the
    DVE multiply, eliminating half of the baseline's per-pair gathers.
  - dma_gather needs int16 indices into a single offset-0 source. Rather
    than range-bucketing the vocab (which forces stream sorting and
    padding), each 512-row sub-batch gets a host-packed DEDUP'd table of
    the distinct out_embed rows it references (~26.5k < 32767 rows), so
    every block gather is one single-source call with zero padding.
  - Hierarchy pairs use a per-core packed in_embed table (<=1024 rows);
    one 1024-row gather yields both sides pair-aligned.
  - Per-block compute: prod = R * I[:,b,:] (bf16, broadcast), add
    halves, reduce -> dots [128, 16, 60] f32. Tail: softplus via
    max(v,0)+ln(1+exp(-|v|)) with the pos/neg sign handled by two
    tensor_scalar variants on slot ranges; hier slots: sub, square,
    reduce. Output per core: [128, 2] partial sums, summed on host in
    f64. No padding corrections needed: every gathered pair is real.
"""

import numpy as np
import ml_dtypes

import concourse.bacc as bacc
import concourse.tile as tile
from concourse import mybir

# Problem shape (hardcoded per contract).
B = 16384
V = 100000
D = 128
C = 10
NEG = 50
PH = 4096
NCORES = 8
P = 128

BL = B // NCORES          # 2048 batch rows per core
HLC = PH // NCORES        # 512 hierarchy pairs per core
NBLK = BL // P            # 16 blocks of 128 rows
SLOTS = C + NEG           # 60 pairs per batch row
SUB = 512                 # rows per dedup sub-batch (keeps tables < int16)
NSUB = BL // SUB          # 4 sub-batches per core
BPS = SUB // P            # 4 blocks per sub-batch
NQ = 4                    # SWDGE queues
GSPLIT = 4                # gathers per block (queue parallelism)

BF16 = mybir.dt.bfloat16
F32 = mybir.dt.float32
I16 = mybir.dt.int16



FP8 = mybir.dt.float8e4
FP8_SCALE = 2048.0


def _dma_gather_fp8(nc, out_ap, in_ap, idxs_ap, num_idxs, queue_num):
    """dma_gather with 128-byte fp8 elements (table rows padded to 256B).

    Clone of BassGpSimd.dma_gather's non-transpose lowering without the
    elem_size_bytes % 256 assert: the non-transpose SWDGE ucode builds one
    descriptor of elem_size_bytes per index with a 256B-granular source
    stride, so 128B payloads are legal. in_ap must be a [rows, 128] view
    of a [rows, 256] fp8 table (stride 256 elems); idx i then fetches the
    128B payload of row i.
    """
    eng = nc.gpsimd
    elem_size = 128          # fp8 elements = bytes
    stride_bytes_256 = 1     # 256B row stride
    _in_ap = eng.lower_ap_dma(in_ap, for_custom_bir_dma=True)
    _idxs_ap = eng.lower_ap(idxs_ap)
    _out_ap = eng.lower_ap(out_ap)
    return eng.add_instruction(
        mybir.InstDMAGatherAnt(
            name=nc.get_next_instruction_name()
            if hasattr(nc, "get_next_instruction_name")
            else eng.bass.get_next_instruction_name(),
            ins=[
                *_in_ap,
                _idxs_ap,
                eng.lower_val_access(eng.to_reg(num_idxs)),
            ],
            outs=[_out_ap],
            transpose=False,
            num_idxs=num_idxs,
            elem_size=elem_size,
            stride_bytes_256=stride_bytes_256,
            gen_mode=0,
            single_packet=False,
            queue_num=queue_num,
            sbuf_tokens_per_rank=0,
            sbuf_free_dim_per_rank=0,
            sbuf_free_dim_pad_per_rank=0,
            sbuf_byte_offset=0,
        )
    )


def _wrap_idx(flat):
    """Index stream -> [128, n/16] int16 tile (16-partition wrap, x8)."""
    return np.tile(flat.astype(np.int16).reshape(-1, 16).T, (8, 1)).copy()


def make_plan(input_labels, pos_labels, neg_labels, hierarchy_pairs,
              w_in, w_out):
    fp8_np = mybir.dt.np(FP8)
    il = np.asarray(input_labels).astype(np.int64)
    pl = np.asarray(pos_labels).astype(np.int64)
    nl = np.asarray(neg_labels).astype(np.int64)
    hp = np.asarray(hierarchy_pairs).astype(np.int64)

    rlab = np.concatenate([pl, nl], axis=1)  # [B, 60]

    uniq = []  # [(core, sub)] -> unique label array
    for k in range(NCORES):
        for s in range(NSUB):
            r0 = k * BL + s * SUB
            uniq.append(np.unique(rlab[r0 : r0 + SUB]))
    capu = -(-max(len(u) for u in uniq) // 16) * 16

    per_core = []
    for k in range(NCORES):
        rows = slice(k * BL, (k + 1) * BL)
        iu, iinv = np.unique(il[rows], return_inverse=True)
        itab = np.zeros((BL, D), ml_dtypes.bfloat16)
        itab[: len(iu)] = w_in[iu]

        rtabs = []
        ridx = np.empty((BL, SLOTS), np.int16)
        for s in range(NSUB):
            u = uniq[k * NSUB + s]
            inv = np.searchsorted(u, rlab[k * BL + s * SUB : k * BL + (s + 1) * SUB])
            rt = np.zeros((capu, D), ml_dtypes.bfloat16)
            rt[: len(u)] = w_out[u]
            rtabs.append(rt)
            ridx[s * SUB : (s + 1) * SUB] = inv

        hl = hp[k * HLC : (k + 1) * HLC]  # [512, 2]
        hu, hinv = np.unique(hl, return_inverse=True)
        hinv = hinv.reshape(HLC, 2)
        htab = np.zeros((1024, D), ml_dtypes.bfloat16)
        htab[: len(hu)] = w_in[hu]

        # right stream: g = b*7680 + j*128 + p  ->  ridx[b*128+p, j]
        rstream = ridx.reshape(NBLK, P, SLOTS).transpose(0, 2, 1).reshape(-1)
        hstream = np.concatenate([hinv[:, 0], hinv[:, 1]])

        rt8s = []
        for s in range(NSUB):
            t8 = np.zeros((capu, 2 * D), fp8_np)
            t8[:, :D] = (rtabs[s].astype(np.float32) * FP8_SCALE).astype(fp8_np)
            rt8s.append(t8)
        per_core.append({
            **{f"rt{s}": rtabs[s] for s in range(NSUB)},
            **{f"rp{s}": rtabs[s].reshape(capu // 2, 2 * D) for s in range(NSUB)},
            **{f"rt8_{s}": rt8s[s] for s in range(NSUB)},
            "itab": itab,
            "htab": htab,
            "ridx": _wrap_idx(rstream),
            "ridx2": _wrap_idx(rstream // 2),
            "iidx": _wrap_idx(iinv),
            "hidx": _wrap_idx(hstream),
        })
    return capu, per_core


def build_program(capu, enable_asserts=False, repeat=1, parts="all",
                  single_packet=False, gsplit=GSPLIT, elem512=False, nq=NQ,
                  fp8=False):
    do_dma = parts in ("all", "dma")
    do_compute = parts in ("all", "compute")
    nc = bacc.Bacc(
        "TRN2",
        target_bir_lowering=False,
        debug=False,
        enable_asserts=enable_asserts,
        num_devices=NCORES,
        num_swdge_queues=nq,
    )

    rt = [
        nc.dram_tensor(f"rt{s}", [capu, D], BF16, kind="ExternalInput").ap()
        for s in range(NSUB)
    ]
    if fp8:
        rt8 = [
            nc.dram_tensor(f"rt8_{s}", [capu, 2 * D], FP8,
                           kind="ExternalInput").ap()
            for s in range(NSUB)
        ]
    if elem512:
        rp = [
            nc.dram_tensor(f"rp{s}", [capu // 2, 2 * D], BF16,
                           kind="ExternalInput").ap()
            for s in range(NSUB)
        ]
        ridx2_d = nc.dram_tensor("ridx2", [P, NBLK * SLOTS * P // 16], I16,
                                 kind="ExternalInput").ap()
    itab = nc.dram_tensor("itab", [BL, D], BF16, kind="ExternalInput").ap()
    htab = nc.dram_tensor("htab", [1024, D], BF16, kind="ExternalInput").ap()
    ridx_d = nc.dram_tensor("ridx", [P, NBLK * SLOTS * P // 16], I16,
                            kind="ExternalInput").ap()
    iidx_d = nc.dram_tensor("iidx", [P, BL // 16], I16,
                            kind="ExternalInput").ap()
    hidx_d = nc.dram_tensor("hidx", [P, 2 * HLC // 16], I16,
                            kind="ExternalInput").ap()
    out_d = nc.dram_tensor("out", [P, 2], F32, kind="ExternalOutput").ap()

    IDXB = SLOTS * P // 16  # idx cols per block (480)

    with tile.TileContext(nc) as tc:
        with (
            tc.tile_pool(name="idx", bufs=2) as idxp,
            tc.tile_pool(name="inp", bufs=2) as inpp,
            tc.tile_pool(name="gath", bufs=3) as gp,
            tc.tile_pool(name="prod", bufs=2) as prodp,
            tc.tile_pool(name="s1", bufs=2) as s1p,
            tc.tile_pool(name="dots", bufs=2) as dotsp,
            tc.tile_pool(name="end", bufs=2) as endp,
        ):
          for _rep in range(repeat):
            ridx = idxp.tile([P, NBLK * IDXB], I16, tag="ridx")
            nc.sync.dma_start(ridx[:], ridx_d)
            if elem512:
                ridx2 = idxp.tile([P, NBLK * IDXB], I16, tag="ridx2")
                nc.sync.dma_start(ridx2[:], ridx2_d)
            iidx = idxp.tile([P, BL // 16], I16, tag="iidx")
            nc.sync.dma_start(iidx[:], iidx_d)
            hidx = idxp.tile([P, 2 * HLC // 16], I16, tag="hidx")
            nc.sync.dma_start(hidx[:], hidx_d)

            itile = inpp.tile([P, NBLK, D], BF16, tag="itile")
            htile = inpp.tile([P, 2 * HLC // P, D], BF16, tag="htile")
            if do_dma:
                nc.gpsimd.dma_gather(
                    itile[:], itab, iidx[:], BL, BL, D,
                    single_packet=False, queue_num=0,
                )

            dots = dotsp.tile([P, NBLK, SLOTS], F32, tag="dots")
            qctr = [2]

            for b in range(NBLK):
                s = b // BPS
                g = gp.tile([P, SLOTS, D], BF16, tag="g")
                ns = SLOTS // gsplit
                if do_dma and elem512:
                    # timing probe: half the descriptors, 512B elems
                    # (wrong data: fetches rows [u, u+1] via elem_step)
                    g2 = gp.tile([P, SLOTS // 2, 2 * D], BF16, tag="g2")
                    for h in range(gsplit):
                        nidx = ns * P // 2
                        nc.gpsimd.dma_gather(
                            g2[:, h * ns // 2 : (h + 1) * ns // 2, :],
                            rp[s],
                            ridx2[:, b * IDXB + h * ns * 8 : b * IDXB + h * ns * 8 + nidx // 16],
                            nidx, nidx, 2 * D,
                            single_packet=single_packet,
                            queue_num=qctr[0] % nq,
                        )
                        qctr[0] += 1
                elif do_dma and fp8:
                    g8 = gp.tile([P, SLOTS, D], FP8, tag="g8")
                    for h in range(gsplit):
                        _dma_gather_fp8(
                            nc,
                            g8[:, h * ns : (h + 1) * ns, :],
                            rt8[s][:, 0:D],
                            ridx[:, b * IDXB + h * ns * 8 : b * IDXB + (h + 1) * ns * 8],
                            ns * P,
                            queue_num=qctr[0] % nq,
                        )
                        qctr[0] += 1
                    if do_compute:
                        nc.scalar.activation(
                            out=g[:], in_=g8[:],
                            func=mybir.ActivationFunctionType.Copy,
                        )
                elif do_dma:
                  for h in range(gsplit):
                    nc.gpsimd.dma_gather(
                        g[:, h * ns : (h + 1) * ns, :],
                        rt[s],
                        ridx[:, b * IDXB + h * ns * 8 : b * IDXB + (h + 1) * ns * 8],
                        ns * P, ns * P, D,
                        single_packet=single_packet,
                        queue_num=qctr[0] % nq,
                    )
                    qctr[0] += 1
                if not do_compute:
                    continue
                prod = prodp.tile([P, SLOTS, D], BF16, tag="prod")
                nc.vector.tensor_tensor(
                    out=prod[:],
                    in0=g[:],
                    in1=itile[:, b : b + 1, :].broadcast_to([P, SLOTS, D]),
                    op=mybir.AluOpType.mult,
                )
                nc.vector.reduce_sum(
                    out=dots[:, b, :], in_=prod[:], axis=mybir.AxisListType.X
                )

            if do_dma:
                nc.gpsimd.dma_gather(
                    htile[:], htab, hidx[:], 2 * HLC, 2 * HLC, D,
                    single_packet=False, queue_num=1,
                )
            if not do_compute:
                out_sb = endp.tile([P, 2], F32, tag="out_sb")
                nc.vector.memset(out_sb[:], 0.0)
                nc.sync.dma_start(out_d, out_sb[:])
                continue
            # hierarchy: htile slots 0:4 = left rows, 4:8 = right rows
            nh = HLC // P  # 4
            dif = endp.tile([P, nh, D], BF16, tag="dif")
            nc.vector.tensor_tensor(
                out=dif[:], in0=htile[:, 0:nh, :], in1=htile[:, nh : 2 * nh, :],
                op=mybir.AluOpType.subtract,
            )
            sq = endp.tile([P, nh, D], F32, tag="sq")
            nc.scalar.activation(
                out=sq[:], in_=dif[:],
                func=mybir.ActivationFunctionType.Square,
            )
            h_acc = endp.tile([P, 1], F32, tag="h_acc")
            nc.vector.reduce_sum(out=h_acc[:], in_=sq[:], axis=mybir.AxisListType.XY)

            # softplus(v) = max(v,0) + ln(1+exp(-|v|));
            # v = -dot for pos slots (j<10), +dot for neg slots.
            dsc = 1.0 / FP8_SCALE if fp8 else 1.0
            eb = endp.tile([P, 2, NBLK, SLOTS], F32, tag="eb")
            nc.vector.tensor_scalar(
                out=eb[:, 0, :, 0:C], in0=dots[:, :, 0:C],
                scalar1=0.0, scalar2=-dsc,
                op0=mybir.AluOpType.min, op1=mybir.AluOpType.mult,
            )
            neg_kw = (
                dict(scalar2=dsc, op1=mybir.AluOpType.mult) if fp8
                else dict(scalar2=None)
            )
            nc.vector.tensor_scalar(
                out=eb[:, 0, :, C:SLOTS], in0=dots[:, :, C:SLOTS],
                scalar1=0.0, op0=mybir.AluOpType.max, **neg_kw,
            )
            absv = endp.tile([P, NBLK, SLOTS], F32, tag="absv")
            nc.scalar.activation(
                out=absv[:], in_=dots[:],
                func=mybir.ActivationFunctionType.Abs,
            )
            expv = endp.tile([P, NBLK, SLOTS], F32, tag="expv")
            nc.scalar.activation(
                out=expv[:], in_=absv[:],
                func=mybir.ActivationFunctionType.Exp, scale=-dsc,
            )
            nc.scalar.activation(
                out=eb[:, 1, :, :], in_=expv[:],
                func=mybir.ActivationFunctionType.Ln, bias=1.0,
            )
            r1 = endp.tile([P, 2, NBLK], F32, tag="r1")
            nc.vector.reduce_sum(out=r1[:], in_=eb[:], axis=mybir.AxisListType.X)
            s_acc = endp.tile([P, 1], F32, tag="s_acc")
            nc.vector.reduce_sum(out=s_acc[:], in_=r1[:], axis=mybir.AxisListType.XY)

            out_sb = endp.tile([P, 2], F32, tag="out_sb")
            nc.vector.tensor_copy(out_sb[:, 0:1], s_acc[:])
            nc.vector.tensor_copy(out_sb[:, 1:2], h_acc[:])
            nc.sync.dma_start(out_d, out_sb[:])

    nc.compile()
    return nc


def prepare(input_labels, pos_labels, neg_labels, hierarchy_pairs,
            in_embed_w, out_embed_w):
    w_in = np.asarray(in_embed_w, dtype=np.float32).astype(ml_dtypes.bfloat16)
    w_out = np.asarray(out_embed_w, dtype=np.float32).astype(ml_dtypes.bfloat16)

    capu, per_core = make_plan(input_labels, pos_labels, neg_labels,
                               hierarchy_pairs, w_in, w_out)
    nc = build_program(capu)
    return nc, per_core, None


def combine_results(per_core_outs, pads):
    s_total = 0.0
    h_total = 0.0
    for r in per_core_outs:
        o = r["out"].astype(np.float64)
        s_total += o[:, 0].sum()
        h_total += o[:, 1].sum()
    loss_graph = s_total / B
    loss_h = 0.5 * 1e-8 * h_total
    return (np.float32(loss_graph + loss_h), np.float32(loss_h))


def run_on_hw(nc, in_maps, **kwargs):
    from concourse.bass_utils import run_bass_kernel_spmd

    return run_bass_kernel_spmd(
        nc, in_maps, core_ids=list(range(NCORES)), **kwargs
    )


def kernel(input_labels, pos_labels, neg_labels, hierarchy_pairs,
           in_embed_w, out_embed_w):
    nc, in_maps, pads = prepare(
        input_labels, pos_labels, neg_labels, hierarchy_pairs,
        in_embed_w, out_embed_w,
    )
    res = run_on_hw(nc, in_maps)
    return combine_results(res.results, pads)


# revision 21
# speedup vs baseline: 3.0214x; 3.0214x over previous
"""Skip-gram negative-sampling loss on 8 Trainium2 NeuronCores.

Strategy v2 (data-parallel over batch, hint-conformant):
  - Each core handles 2048 batch rows and 512 hierarchy pairs.
  - Row-major pair layout: block b (of 16) covers batch rows
    [128b, 128b+128); pair (row, j) sits at partition row%128, slot j
    (j<10: pos, j>=10: neg). The input-row operand is gathered ONCE per
    core (2048 rows) and broadcast (stride-0) across the 60 slots by the
    DVE multiply, eliminating half of the baseline's per-pair gathers.
  - dma_gather needs int16 indices into a single offset-0 source. Rather
    than range-bucketing the vocab (which forces stream sorting and
    padding), each 512-row sub-batch gets a host-packed DEDUP'd table of
    the distinct out_embed rows it references (~26.5k < 32767 rows), so
    every block gather is one single-source call with zero padding.
  - Hierarchy pairs use a per-core packed in_embed table (<=1024 rows);
    one 1024-row gather yields both sides pair-aligned.
  - Per-block compute: prod = R * I[:,b,:] (bf16, broadcast), add
    halves, reduce -> dots [128, 16, 60] f32. Tail: softplus via
    max(v,0)+ln(1+exp(-|v|)) with the pos/neg sign handled by two
    tensor_scalar variants on slot ranges; hier slots: sub, square,
    reduce. Output per core: [128, 2] partial sums, summed on host in
    f64. No padding corrections needed: every gathered pair is real.
"""

import numpy as np
import ml_dtypes

import concourse.bacc as bacc
import concourse.tile as tile
from concourse import mybir

# Problem shape (hardcoded per contract).
B = 16384
LAST_CAPU = None
V = 100000
D = 128
C = 10
NEG = 50
PH = 4096
NCORES = 8
P = 128

BL = B // NCORES          # 2048 batch rows per core
HLC = PH // NCORES        # 512 hierarchy pairs per core
NBLK = BL // P            # 16 blocks of 128 rows
SLOTS = C + NEG           # 60 pairs per batch row
SUB = 512                 # rows per dedup sub-batch (keeps tables < int16)
NSUB = BL // SUB          # 4 sub-batches per core
BPS = SUB // P            # 4 blocks per sub-batch
NQ = 4                    # SWDGE queues
GSPLIT = 4                # gathers per block (queue parallelism)

BF16 = mybir.dt.bfloat16
F32 = mybir.dt.float32
I16 = mybir.dt.int16



FP8 = mybir.dt.float8e4
FP8_SCALE = 2048.0


def _dma_gather_fp8(nc, out_ap, in_ap, idxs_ap, num_idxs, queue_num):
    """dma_gather with 128-byte fp8 elements (table rows padded to 256B).

    Clone of BassGpSimd.dma_gather's non-transpose lowering without the
    elem_size_bytes % 256 assert: the non-transpose SWDGE ucode builds one
    descriptor of elem_size_bytes per index with a 256B-granular source
    stride, so 128B payloads are legal. in_ap must be a [rows, 128] view
    of a [rows, 256] fp8 table (stride 256 elems); idx i then fetches the
    128B payload of row i.
    """
    eng = nc.gpsimd
    elem_size = 128          # fp8 elements = bytes
    stride_bytes_256 = 1     # 256B row stride
    _in_ap = eng.lower_ap_dma(in_ap, for_custom_bir_dma=True)
    _idxs_ap = eng.lower_ap(idxs_ap)
    _out_ap = eng.lower_ap(out_ap)
    return eng.add_instruction(
        mybir.InstDMAGatherAnt(
            name=nc.get_next_instruction_name()
            if hasattr(nc, "get_next_instruction_name")
            else eng.bass.get_next_instruction_name(),
            ins=[
                *_in_ap,
                _idxs_ap,
                eng.lower_val_access(eng.to_reg(num_idxs)),
            ],
            outs=[_out_ap],
            transpose=False,
            num_idxs=num_idxs,
            elem_size=elem_size,
            stride_bytes_256=stride_bytes_256,
            gen_mode=0,
            single_packet=False,
            queue_num=queue_num,
            sbuf_tokens_per_rank=0,
            sbuf_free_dim_per_rank=0,
            sbuf_free_dim_pad_per_rank=0,
            sbuf_byte_offset=0,
        )
    )


def _wrap_idx(flat):
    """Index stream -> [128, n/16] int16 tile (16-partition wrap, x8)."""
    return np.tile(flat.astype(np.int16).reshape(-1, 16).T, (8, 1)).copy()


def make_plan(input_labels, pos_labels, neg_labels, hierarchy_pairs,
              w_in, w_out):
    il = np.asarray(input_labels).astype(np.int64)
    pl = np.asarray(pos_labels).astype(np.int64)
    nl = np.asarray(neg_labels).astype(np.int64)
    hp = np.asarray(hierarchy_pairs).astype(np.int64)

    rlab = np.concatenate([pl, nl], axis=1)  # [B, 60]

    uniq = []  # [(core, sub)] -> unique label array
    for k in range(NCORES):
        for s in range(NSUB):
            r0 = k * BL + s * SUB
            uniq.append(np.unique(rlab[r0 : r0 + SUB]))
    capu = -(-max(len(u) for u in uniq) // 16) * 16

    per_core = []
    for k in range(NCORES):
        rows = slice(k * BL, (k + 1) * BL)
        iu, iinv = np.unique(il[rows], return_inverse=True)
        itab = np.zeros((BL, D), ml_dtypes.bfloat16)
        itab[: len(iu)] = w_in[iu]

        rtabs = []
        ridx = np.empty((BL, SLOTS), np.int16)
        for s in range(NSUB):
            u = uniq[k * NSUB + s]
            inv = np.searchsorted(u, rlab[k * BL + s * SUB : k * BL + (s + 1) * SUB])
            rt = np.zeros((capu, D), ml_dtypes.bfloat16)
            rt[: len(u)] = w_out[u]
            rtabs.append(rt)
            ridx[s * SUB : (s + 1) * SUB] = inv

        hl = hp[k * HLC : (k + 1) * HLC]  # [512, 2]
        hu, hinv = np.unique(hl, return_inverse=True)
        hinv = hinv.reshape(HLC, 2)
        htab = np.zeros((1024, D), ml_dtypes.bfloat16)
        htab[: len(hu)] = w_in[hu]

        # right stream: g = b*7680 + j*128 + p  ->  ridx[b*128+p, j]
        rstream = ridx.reshape(NBLK, P, SLOTS).transpose(0, 2, 1).reshape(-1)
        hstream = np.concatenate([hinv[:, 0], hinv[:, 1]])

        per_core.append({
            **{f"rt{s}": rtabs[s] for s in range(NSUB)},
            "itab": itab,
            "htab": htab,
            "ridx": _wrap_idx(rstream),
            "iidx": _wrap_idx(iinv),
            "hidx": _wrap_idx(hstream),
        })
    return capu, per_core


def build_program(capu, enable_asserts=False, repeat=1, parts="all",
                  single_packet=False, gsplit=GSPLIT, elem512=False, nq=NQ,
                  fp8=False, gbufs=3, pbufs=2):
    do_dma = parts in ("all", "dma")
    do_compute = parts in ("all", "compute")
    nc = bacc.Bacc(
        "TRN2",
        target_bir_lowering=False,
        debug=False,
        enable_asserts=enable_asserts,
        num_devices=NCORES,
        num_swdge_queues=nq,
    )

    rt = [
        nc.dram_tensor(f"rt{s}", [capu, D], BF16, kind="ExternalInput").ap()
        for s in range(NSUB)
    ]
    if fp8:
        rt8 = [
            nc.dram_tensor(f"rt8_{s}", [capu, 2 * D], FP8,
                           kind="ExternalInput").ap()
            for s in range(NSUB)
        ]
    if elem512:
        rp = [
            nc.dram_tensor(f"rp{s}", [capu // 2, 2 * D], BF16,
                           kind="ExternalInput").ap()
            for s in range(NSUB)
        ]
        ridx2_d = nc.dram_tensor("ridx2", [P, NBLK * SLOTS * P // 16], I16,
                                 kind="ExternalInput").ap()
    itab = nc.dram_tensor("itab", [BL, D], BF16, kind="ExternalInput").ap()
    htab = nc.dram_tensor("htab", [1024, D], BF16, kind="ExternalInput").ap()
    ridx_d = nc.dram_tensor("ridx", [P, NBLK * SLOTS * P // 16], I16,
                            kind="ExternalInput").ap()
    iidx_d = nc.dram_tensor("iidx", [P, BL // 16], I16,
                            kind="ExternalInput").ap()
    hidx_d = nc.dram_tensor("hidx", [P, 2 * HLC // 16], I16,
                            kind="ExternalInput").ap()
    out_d = nc.dram_tensor("out", [P, 2], F32, kind="ExternalOutput").ap()

    IDXB = SLOTS * P // 16  # idx cols per block (480)

    with tile.TileContext(nc) as tc:
        with (
            tc.tile_pool(name="idx", bufs=2) as idxp,
            tc.tile_pool(name="inp", bufs=2) as inpp,
            tc.tile_pool(name="gath", bufs=gbufs) as gp,
            tc.tile_pool(name="prod", bufs=pbufs) as prodp,
            tc.tile_pool(name="s1", bufs=2) as s1p,
            tc.tile_pool(name="dots", bufs=2) as dotsp,
            tc.tile_pool(name="end", bufs=2) as endp,
        ):
          for _rep in range(repeat):
            ridx = idxp.tile([P, NBLK * IDXB], I16, tag="ridx")
            nc.sync.dma_start(ridx[:], ridx_d)
            if elem512:
                ridx2 = idxp.tile([P, NBLK * IDXB], I16, tag="ridx2")
                nc.sync.dma_start(ridx2[:], ridx2_d)
            iidx = idxp.tile([P, BL // 16], I16, tag="iidx")
            nc.sync.dma_start(iidx[:], iidx_d)
            hidx = idxp.tile([P, 2 * HLC // 16], I16, tag="hidx")
            nc.sync.dma_start(hidx[:], hidx_d)

            itile = inpp.tile([P, NBLK, D], BF16, tag="itile")
            htile = inpp.tile([P, 2 * HLC // P, D], BF16, tag="htile")
            if do_dma:
                nc.gpsimd.dma_gather(
                    itile[:], itab, iidx[:], BL, BL, D,
                    single_packet=False, queue_num=0,
                )
                nc.gpsimd.dma_gather(
                    htile[:], htab, hidx[:], 2 * HLC, 2 * HLC, D,
                    single_packet=False, queue_num=1,
                )

            dots = dotsp.tile([P, NBLK, SLOTS], F32, tag="dots")
            qctr = [2]

            for b in range(NBLK):
                s = b // BPS
                g = gp.tile([P, SLOTS, D], BF16, tag="g")
                ns = SLOTS // gsplit
                if do_dma and elem512:
                    # timing probe: half the descriptors, 512B elems
                    # (wrong data: fetches rows [u, u+1] via elem_step)
                    g2 = gp.tile([P, SLOTS // 2, 2 * D], BF16, tag="g2")
                    for h in range(gsplit):
                        nidx = ns * P // 2
                        nc.gpsimd.dma_gather(
                            g2[:, h * ns // 2 : (h + 1) * ns // 2, :],
                            rp[s],
                            ridx2[:, b * IDXB + h * ns * 8 : b * IDXB + h * ns * 8 + nidx // 16],
                            nidx, nidx, 2 * D,
                            single_packet=single_packet,
                            queue_num=qctr[0] % nq,
                        )
                        qctr[0] += 1
                elif do_dma and fp8:
                    g8 = gp.tile([P, SLOTS, D], FP8, tag="g8")
                    for h in range(gsplit):
                        _dma_gather_fp8(
                            nc,
                            g8[:, h * ns : (h + 1) * ns, :],
                            rt8[s][:, 0:D],
                            ridx[:, b * IDXB + h * ns * 8 : b * IDXB + (h + 1) * ns * 8],
                            ns * P,
                            queue_num=qctr[0] % nq,
                        )
                        qctr[0] += 1
                    if do_compute:
                        nc.scalar.activation(
                            out=g[:], in_=g8[:],
                            func=mybir.ActivationFunctionType.Copy,
                        )
                elif do_dma:
                  for h in range(gsplit):
                    nc.gpsimd.dma_gather(
                        g[:, h * ns : (h + 1) * ns, :],
                        rt[s],
                        ridx[:, b * IDXB + h * ns * 8 : b * IDXB + (h + 1) * ns * 8],
                        ns * P, ns * P, D,
                        single_packet=single_packet,
                        queue_num=qctr[0] % nq,
                    )
                    qctr[0] += 1
                if not do_compute:
                    continue
                prod = prodp.tile([P, SLOTS, D], BF16, tag="prod")
                nc.vector.tensor_tensor(
                    out=prod[:],
                    in0=g[:],
                    in1=itile[:, b : b + 1, :].broadcast_to([P, SLOTS, D]),
                    op=mybir.AluOpType.mult,
                )
                nc.vector.reduce_sum(
                    out=dots[:, b, :], in_=prod[:], axis=mybir.AxisListType.X
                )

            if not do_compute:
                out_sb = endp.tile([P, 2], F32, tag="out_sb")
                nc.vector.memset(out_sb[:], 0.0)
                nc.sync.dma_start(out_d, out_sb[:])
                continue
            # hierarchy: htile slots 0:4 = left rows, 4:8 = right rows
            nh = HLC // P  # 4
            dif = endp.tile([P, nh, D], BF16, tag="dif")
            nc.vector.tensor_tensor(
                out=dif[:], in0=htile[:, 0:nh, :], in1=htile[:, nh : 2 * nh, :],
                op=mybir.AluOpType.subtract,
            )
            sq = endp.tile([P, nh, D], F32, tag="sq")
            nc.scalar.activation(
                out=sq[:], in_=dif[:],
                func=mybir.ActivationFunctionType.Square,
            )
            h_acc = endp.tile([P, 1], F32, tag="h_acc")
            nc.vector.reduce_sum(out=h_acc[:], in_=sq[:], axis=mybir.AxisListType.XY)

            # softplus(v) = max(v,0) + ln(1+exp(-|v|));
            # v = -dot for pos slots (j<10), +dot for neg slots.
            dsc = 1.0 / FP8_SCALE if fp8 else 1.0
            eb = endp.tile([P, 2, NBLK, SLOTS], F32, tag="eb")
            nc.vector.tensor_scalar(
                out=eb[:, 0, :, 0:C], in0=dots[:, :, 0:C],
                scalar1=0.0, scalar2=-dsc,
                op0=mybir.AluOpType.min, op1=mybir.AluOpType.mult,
            )
            neg_kw = (
                dict(scalar2=dsc, op1=mybir.AluOpType.mult) if fp8
                else dict(scalar2=None)
            )
            nc.vector.tensor_scalar(
                out=eb[:, 0, :, C:SLOTS], in0=dots[:, :, C:SLOTS],
                scalar1=0.0, op0=mybir.AluOpType.max, **neg_kw,
            )
            absv = endp.tile([P, NBLK, SLOTS], F32, tag="absv")
            nc.scalar.activation(
                out=absv[:], in_=dots[:],
                func=mybir.ActivationFunctionType.Abs,
            )
            expv = endp.tile([P, NBLK, SLOTS], F32, tag="expv")
            nc.scalar.activation(
                out=expv[:], in_=absv[:],
                func=mybir.ActivationFunctionType.Exp, scale=-dsc,
            )
            nc.scalar.activation(
                out=eb[:, 1, :, :], in_=expv[:],
                func=mybir.ActivationFunctionType.Ln, bias=1.0,
            )
            r1 = endp.tile([P, 2, NBLK], F32, tag="r1")
            nc.vector.reduce_sum(out=r1[:], in_=eb[:], axis=mybir.AxisListType.X)
            s_acc = endp.tile([P, 1], F32, tag="s_acc")
            nc.vector.reduce_sum(out=s_acc[:], in_=r1[:], axis=mybir.AxisListType.XY)

            out_sb = endp.tile([P, 2], F32, tag="out_sb")
            nc.vector.tensor_copy(out_sb[:, 0:1], s_acc[:])
            nc.vector.tensor_copy(out_sb[:, 1:2], h_acc[:])
            nc.sync.dma_start(out_d, out_sb[:])

    nc.compile()
    return nc


def prepare(input_labels, pos_labels, neg_labels, hierarchy_pairs,
            in_embed_w, out_embed_w):
    w_in = np.asarray(in_embed_w, dtype=np.float32).astype(ml_dtypes.bfloat16)
    w_out = np.asarray(out_embed_w, dtype=np.float32).astype(ml_dtypes.bfloat16)

    capu, per_core = make_plan(input_labels, pos_labels, neg_labels,
                               hierarchy_pairs, w_in, w_out)
    global LAST_CAPU
    LAST_CAPU = capu
    nc = build_program(capu)
    return nc, per_core, None


def combine_results(per_core_outs, pads):
    s_total = 0.0
    h_total = 0.0
    for r in per_core_outs:
        o = r["out"].astype(np.float64)
        s_total += o[:, 0].sum()
        h_total += o[:, 1].sum()
    loss_graph = s_total / B
    loss_h = 0.5 * 1e-8 * h_total
    return (np.float32(loss_graph + loss_h), np.float32(loss_h))


def run_on_hw(nc, in_maps, **kwargs):
    from concourse.bass_utils import run_bass_kernel_spmd

    return run_bass_kernel_spmd(
        nc, in_maps, core_ids=list(range(NCORES)), **kwargs
    )


def kernel(input_labels, pos_labels, neg_labels, hierarchy_pairs,
           in_embed_w, out_embed_w):
    nc, in_maps, pads = prepare(
        input_labels, pos_labels, neg_labels, hierarchy_pairs,
        in_embed_w, out_embed_w,
    )
    res = run_on_hw(nc, in_maps)
    return combine_results(res.results, pads)
